# revision 39
# baseline (speedup 1.0000x reference)
"""Causal self-attention (GQA, rope, qk-rmsnorm) Trainium2 kernel, 8 NeuronCores.

Sharding: core = (b, g), b = core // 4 (batch), g = core % 4.
Row-sharded Q/attention/output (query row-chunks {g, 4+g, 8+g, 12+g} per core)
with HEAD-SHARDED K/V projection: each core computes only kv-head g's K and V
over all T tokens, then the per-batch 4-core groups AllGather K/V.

Optimizations vs the 408us baseline (final: ~377us):
- all large inputs arrive HOST-TILED in the exact SBUF layout, so every DMA
  descriptor is a 2-16KB contiguous run (the baseline's rearranged loads
  generated 256-512B descriptors and halved effective DMA bandwidth)
- attention is a two-engine pipeline at 2-pack lag so the in-order PE
  sequencer never blocks: PE fills packed multi-bank score psum tiles
  (causal mask folded in as an additive -30 bias matmul so exp(S-30)~0),
  ACT exps each 1024-wide pack in ONE instruction (5/head instead of 16,
  killing ~190ns/instr bubbles), then PE drains PV + per-kt denominator
  matmuls two packs later when their exp is long done; the vector engine
  only normalizes each finished head.  PE runs ~95% busy at full clock
  through attention (the p-state ramp penalizes any sub-us stall).
- K/V proj writes K|V for 4 token tiles into one [128,1024] psum quad and
  stages K/V out via the idle ACT engine so psum slots recycle at matmul
  speed; rope/rms/transpose epilogues are deferred one quarter
- ONE merged AllGather (the CC engine cannot begin before ~90us into the
  kernel regardless of data readiness, so splitting only adds mesh
  round-trips and its early unpack descriptors block HWDGE rings that
  phase-1 weight loads need); recv unpack runs on sync HWDGE queues into
  per-kv-head K/V tiles so each head's attention waits only on its own DMA
- x eighth-slabs double-buffered; wq/xoT prefetched under the K/V phase;
  PE warmup transposes ramp the clock while the first loads stream;
  O-projection alternates psum pools so four tiles are in flight;
  output stored bf16 (host casts back to f32)
"""

import sys

if "/opt/trn_rl_repo" not in sys.path:
    sys.path.insert(0, "/opt/trn_rl_repo")

import ml_dtypes
import numpy as np

BF = ml_dtypes.bfloat16

B, T, C = 2, 2048, 2048
NH, NKV = 16, 4
HD = C // NH  # 128
P = 128
NT = T // P            # 16 key token tiles
NCT = C // P           # 16 contraction tiles
QROWS = 512            # own query rows per core
NQT = QROWS // P       # 4 own token tiles
EPS = float(np.finfo(np.float32).eps)

# attention score-tile packing: 5 psum tiles of [128, 1024] per head.
# each entry: (col_offset, kt, n) where n = valid query cols for key tile kt.
# A: kt0,kt1 (n=512); B: kt2,kt3; C: kt4(384),kt12(128),kt5,kt13;
# D: kt6,kt14,kt7,kt15; E: kt8..kt11 (n=256).  All matmul writes stay inside
# one 2KB psum bank; each pack is exp'd by a single 1024-wide ACT op.
PACK = [
    [(0, 0, 512), (512, 1, 512)],
    [(0, 2, 512), (512, 3, 512)],
    [(0, 4, 384), (384, 12, 128), (512, 5, 384), (896, 13, 128)],
    [(0, 6, 384), (384, 14, 128), (512, 7, 384), (896, 15, 128)],
    [(0, 8, 256), (256, 9, 256), (512, 10, 256), (768, 11, 256)],
]

_CACHE = {}


def _chunks(g):
    return [g, 4 + g, 8 + g, 12 + g]


def _rows(g):
    return np.concatenate([np.arange(ch * P, (ch + 1) * P) for ch in _chunks(g)])


def _qbias_t(g):
    """Additive causal bias, layout [k i, slot c, sub s, q j] (partition-major).

    For slot c the score tile is S^T[k, q] with q the 128 rows of chunk 4c+g.
    bias = 0 if key (512*c + s*128 + i) <= query (128*(4c+g) + j) else -30.
    Accumulated into the S psum by a PE matmul with an identity stationary,
    so exp(S - 30) ~ 1e-13 kills masked keys with no vector-engine work.
    """
    m = np.zeros((4, 4, P, P), np.float32)
    for c in range(4):
        k = 512 * c + np.arange(512)[:, None]          # (512, 1)
        q = (4 * c + g) * P + np.arange(P)[None, :]    # (1, 128)
        m[c] = np.where(k <= q, 0.0, -30.0).reshape(4, P, P)
    return np.ascontiguousarray(m.transpose(2, 0, 1, 3).astype(BF))


def _build():
    import concourse.bacc as bacc
    import concourse.mybir as mybir
    import concourse.tile as tile
    from concourse.masks import make_identity

    f32 = mybir.dt.float32
    bf16 = mybir.dt.bfloat16
    AF = mybir.ActivationFunctionType
    OP = mybir.AluOpType
    AX = mybir.AxisListType

    nc = bacc.Bacc("TRN2", target_bir_lowering=False, debug=False, num_devices=8)

    # host-tiled inputs: leading dim = (block, partition) so every DMA is a
    # [128, ...] slice with a long contiguous run per partition.
    xet = nc.dram_tensor("xet", [8 * 4 * P, 4, 256], bf16, kind="ExternalInput").ap()
    xot = nc.dram_tensor("xot", [4 * P, 4, QROWS], bf16, kind="ExternalInput").ap()
    wqt = nc.dram_tensor("wqt", [4 * P, NCT, 512], bf16, kind="ExternalInput").ap()
    wot = nc.dram_tensor("wot", [4 * P, NCT, 512], bf16, kind="ExternalInput").ap()
    wkvt = nc.dram_tensor("wkvt", [P, NCT, 256], bf16, kind="ExternalInput").ap()
    cosft = nc.dram_tensor("cosft", [4 * P, 4, 64], f32, kind="ExternalInput").ap()
    sinft = nc.dram_tensor("sinft", [4 * P, 4, 64], f32, kind="ExternalInput").ap()
    cosot = nc.dram_tensor("cosot", [4 * P, 4, 64], f32, kind="ExternalInput").ap()
    sinot = nc.dram_tensor("sinot", [4 * P, 4, 64], f32, kind="ExternalInput").ap()
    qmt = nc.dram_tensor("qmt", [P, 4, 4, P], bf16, kind="ExternalInput").ap()
    yo = nc.dram_tensor("yo", [QROWS, C], bf16, kind="ExternalOutput").ap()

    with tile.TileContext(nc) as tc:
        with (
            tc.tile_pool(name="singles", bufs=1) as singles,
            tc.tile_pool(name="big", bufs=1) as bigpool,
            tc.tile_pool(name="xq", bufs=2) as xqpool,
            tc.tile_pool(name="slab", bufs=3) as slabpool,
            tc.tile_pool(name="cs", bufs=2) as cspool,
            tc.tile_pool(name="epi", bufs=1) as epipool,
            tc.tile_pool(name="qh", bufs=2) as qhpool,
            tc.tile_pool(name="pt", bufs=5) as ptpool,
            tc.tile_pool(name="smallf", bufs=2) as smallf,
            tc.tile_pool(name="stg", bufs=3) as stgpool,
            tc.tile_pool(name="outs", bufs=2) as outpool,
            tc.tile_pool(name="dram", bufs=1, space="DRAM") as drampool,
            tc.tile_pool(name="psS", bufs=2, space="PSUM") as psS,
            tc.tile_pool(name="psY", bufs=2, space="PSUM") as psY,
            tc.tile_pool(name="psD", bufs=2, space="PSUM") as psD,
        ):
            # ---- first: the loads the first matmuls depend on ----
            def load_slab(w_ap, s, name):
                """Weight slab s as [128, 16, 512] bf16 in ONE DMA (16KB/row).

                wkv/wq/wo slabs share one 3-slot pool; phases are disjoint so
                the slots time-share SBUF."""
                wsl = slabpool.tile([P, NCT, 512], bf16, tag="slab", name=name)
                nc.sync.dma_start(out=wsl, in_=w_ap[s * P:(s + 1) * P])
                return wsl

            # wkv split into gr chunks, gr0 first: the first K/V matmul
            # (ct=0) needs only wkv-gr0 + x-gr0, so it starts ~4us earlier
            wslkv = slabpool.tile([P, NCT, 512], bf16, tag="slab", name="wkv")
            nc.sync.dma_start(out=wslkv[:, 0:4, 0:256], in_=wkvt[:, 0:4, :])

            def load_xe(e):
                """One x eighth ([C, 256]) as 4 tiles of [P, 4, 256], one DMA
                each (2KB contiguous per partition row)."""
                xh = [xqpool.tile([P, 4, 256], bf16, tag=f"xT{gr}",
                                  name=f"xfT{e}{gr}")
                      for gr in range(4)]
                for gr in range(4):
                    nc.sync.dma_start(
                        out=xh[gr],
                        in_=xet[(e * 4 + gr) * P:(e * 4 + gr + 1) * P])
                return xh

            xh_next = load_xe(0)
            for gr in range(1, 4):
                nc.sync.dma_start(out=wslkv[:, 4 * gr:4 * gr + 4, 0:256],
                                  in_=wkvt[:, 4 * gr:4 * gr + 4, :])

            ident = singles.tile([P, P], bf16)
            make_identity(nc, ident)

            # PE warmup: dummy transposes ramp the tensor-engine p-state while
            # the first x/w DMAs stream in.
            warm = psY.tile([P, P], bf16, tag="Y", name="warm")
            for _ in range(24):
                nc.tensor.transpose(warm, ident, ident)

            ones128 = singles.tile([P, P], bf16)
            nc.vector.memset(ones128, 1.0)
            eps_q = singles.tile([P, 1], f32)
            nc.vector.memset(eps_q, EPS)
            eps_k = singles.tile([P, 1], f32)
            nc.vector.memset(eps_k, HD * EPS)

            # persistent big SBUF tensors
            qT = bigpool.tile([P, NH, QROWS], bf16, tag="qT")      # [d, h, q]
            # per-kv-head K/V tiles: a head's attention only waits for its
            # own head's unpack DMA instead of the whole 4MB
            kTs = [bigpool.tile([P, T], bf16, tag=f"kT{r}", name=f"kT{r}")
                   for r in range(NKV)]
            vAs = [bigpool.tile([P, NT, HD], bf16, tag=f"vA{r}", name=f"vA{r}")
                   for r in range(NKV)]
            yT = bigpool.tile([P, NCT, QROWS], bf16, tag="yT")     # [d, ct, q]
            qmask = singles.tile([P, 4, 4, P], bf16)               # [ki, c, sub, q]
            nc.sync.dma_start(out=qmask, in_=qmt)

            # DRAM bounce buffers for the K/V AllGather. A single collective:
            # the CC engine cannot start before ~90us into the kernel (fixed
            # runtime cadence), by which point all four quarters are ready;
            # splitting also makes early unpack descriptors block the HWDGE
            # rings that phase-1 weight loads need.
            sendh = drampool.tile([P, 4096], bf16, tag="send", name="sendh")
            recvh = drampool.tile([4, P, 4096], bf16, tag="recv", name="recvh")


            # ---------------- helpers ----------------
            pending = []  # delayed PE transpose packs (2-deep pipeline)

            def drain_pending(keep=0):
                while len(pending) > keep:
                    pending.pop(0)()

            def rope_rms(v3, cosn, sinn, out_bf, eps_ap, sqrt_scale, nh):
                """v3: [128, nh, 128] psum f32 view. Writes normalized bf16
                rope output to out_bf [128, nh, 128]."""
                ro = epipool.tile([P, 4, HD], f32, tag="ro", name="ro")[:, 0:nh, :]
                cs = epipool.tile([P, 4, HD], f32, tag="cs", name="cs")[:, 0:nh, :]
                sn = epipool.tile([P, 4, HD], f32, tag="sn", name="sn")[:, 0:nh, :]
                nc.vector.tensor_tensor(cs[:, :, 0:64], v3[:, :, 0:64], cosn, op=OP.mult)
                nc.vector.tensor_tensor(cs[:, :, 64:128], v3[:, :, 64:128], cosn, op=OP.mult)
                nc.vector.tensor_tensor(sn[:, :, 0:64], v3[:, :, 0:64], sinn, op=OP.mult)
                nc.vector.tensor_tensor(sn[:, :, 64:128], v3[:, :, 64:128], sinn, op=OP.mult)
                nc.vector.tensor_tensor(ro[:, :, 0:64], cs[:, :, 0:64], sn[:, :, 64:128], op=OP.add)
                nc.vector.tensor_sub(ro[:, :, 64:128], cs[:, :, 64:128], sn[:, :, 0:64])
                ss = smallf.tile([P, 4], f32, tag="ss", name="ss")[:, 0:nh]
                sq = epipool.tile([P, 4, HD], f32, tag="cs", name="sq")[:, 0:nh, :]
                nc.vector.tensor_tensor(sq, ro, ro, op=OP.mult)
                nc.vector.reduce_sum(ss, sq, axis=AX.X)
                rms = smallf.tile([P, 4], f32, tag="rms", name="rms")[:, 0:nh]
                nc.scalar.activation(rms, ss, AF.Sqrt, bias=eps_ap, scale=sqrt_scale)
                rinv = smallf.tile([P, 4], f32, tag="rms", name="rinv")[:, 0:nh]
                nc.vector.reciprocal_approx_fast(rinv, rms)
                for hh in range(nh):
                    nc.vector.tensor_scalar_mul(
                        out_bf[:, hh, :], ro[:, hh, :], rinv[:, hh:hh + 1]
                    )

            def pack_transpose(src_bf, dst3, nh):
                """src_bf [128, nh, 128] bf16 -> nh PE transposes -> one copy
                to dst3 ([128, nh, 128] view)."""
                ptr = psY.tile([P, 512], bf16, tag="Y", name="ptrq")
                for hh in range(nh):
                    nc.tensor.transpose(
                        ptr[:, hh * P:(hh + 1) * P], src_bf[:, hh, :], ident
                    )
                nc.vector.tensor_copy(
                    dst3, ptr[:, 0:nh * P].rearrange("p (s n) -> p s n", s=nh)
                )

            # ---------------- phase 0: local kv-head K/V projection --------
            def k_epilogue(kstage, quarter):
                # one 4-wide rope+rms+pack for the whole quarter
                cos4q = cspool.tile([P, 4, 64], f32, tag="cs4", name=f"cq{quarter}")
                sin4q = cspool.tile([P, 4, 64], f32, tag="sn4", name=f"sq{quarter}")
                nc.scalar.dma_start(
                    out=cos4q, in_=cosft[quarter * P:(quarter + 1) * P])
                nc.scalar.dma_start(
                    out=sin4q, in_=sinft[quarter * P:(quarter + 1) * P])
                khat4 = qhpool.tile([P, 4, HD], bf16, tag="khat", name=f"kh{quarter}")
                rope_rms(kstage, cos4q, sin4q, khat4, eps_k, 1.0, 4)
                kst4 = stgpool.tile([P, 4, HD], bf16, tag="kst", name=f"kst{quarter}")
                pack_transpose(khat4, kst4, 4)
                nc.sync.dma_start(
                    out=sendh[:, quarter * 512:(quarter + 1) * 512],
                    in_=kst4.rearrange("p a d -> p (a d)"))

            def kv_allgather():
                nc.gpsimd.collective_compute(
                    "AllGather",
                    mybir.AluOpType.bypass,
                    replica_groups=[[0, 1, 2, 3], [4, 5, 6, 7]],
                    ins=[sendh.opt()],
                    outs=[recvh.opt()],
                )
                # recv unpack on the sync HWDGE path (tile inserts the RAW
                # dependency on the collective's completion); per-kv-head
                # tiles so a head's attention waits only on its own unpack.
                for r in range(4):
                    nc.sync.dma_start(out=kTs[r], in_=recvh[r, :, 0:2048])
                    nc.sync.dma_start(
                        out=vAs[r],
                        in_=recvh[r, :, 2048:4096].rearrange(
                            "p (tt d) -> p tt d", tt=NT),
                    )

            # xoT for the Q projection: loaded early, under the K/V phase
            xoT = [
                bigpool.tile([P, 4, QROWS], bf16, tag=f"xoT{gr}", name=f"xoT{gr}")
                for gr in range(4)
            ]

            def load_xo(gr):
                nc.sync.dma_start(out=xoT[gr],
                                  in_=xot[gr * P:(gr + 1) * P])

            wsl_pre = {}
            for quarter in range(4):
                # one [P, 1024] psum quad holds K|V for 4 token tiles; K and V
                # are staged out to SBUF right away so the psum slot recycles
                # at matmul speed (the rope epilogue is deferred a quarter).
                pskv = psS.tile([P, 1024], f32, tag="S", name=f"pskv{quarter}")
                pskv4 = pskv.rearrange("p (t c) -> p t c", t=4)
                kstage = stgpool.tile([P, 4, HD], f32, tag="kstage",
                                      name=f"kstage{quarter}")
                vstage = stgpool.tile([P, 4, HD], bf16, tag="vstage",
                                      name=f"vstage{quarter}")
                for half in range(2):
                    e = 2 * quarter + half
                    xhT = xh_next
                    if e < 7:
                        xh_next = load_xe(e + 1)
                    # spread the phase-1 prefetches across phase-0 order
                    if e == 0:
                        load_xo(0)
                    elif e == 2:
                        load_xo(1)
                    elif e == 3:
                        load_xo(2)
                        wsl_pre[0] = load_slab(wqt, 0, "wq0")
                    elif e == 5:
                        load_xo(3)
                        wsl_pre[1] = load_slab(wqt, 1, "wq1")
                    for tl in range(2 * half, 2 * half + 2):
                        tl2 = tl - 2 * half
                        for ct in range(NCT):
                            nc.tensor.matmul(
                                pskv[:, tl * 256:(tl + 1) * 256],
                                xhT[ct // 4][:, ct % 4, tl2 * P:(tl2 + 1) * P],
                                wslkv[:, ct, 0:256],
                                start=(ct == 0),
                                stop=(ct == NCT - 1),
                                skip_group_check=True,
                            )
                nc.scalar.copy(kstage, pskv4[:, :, 0:HD])
                nc.scalar.copy(vstage, pskv4[:, :, HD:2 * HD])
                nc.sync.dma_start(
                    out=sendh[:, 2048 + quarter * 512:2048 + (quarter + 1) * 512],
                    in_=vstage.rearrange("p a d -> p (a d)"))
                if quarter == 0:
                    k_epilogue(kstage, 0)
                else:
                    drain_pending(1)
                    pending.append(
                        lambda kstage=kstage, quarter=quarter:
                            k_epilogue(kstage, quarter))
            drain_pending()
            kv_allgather()

            # ---------------- phase 1: Q projection ----------------
            def cos_tiles(tt, name):
                cosn = cspool.tile([P, 4, 64], f32, tag="cs4", name=f"c{name}")
                sinn = cspool.tile([P, 4, 64], f32, tag="sn4", name=f"s{name}")
                nc.scalar.dma_start(out=cosn, in_=cosot[tt * P:(tt + 1) * P])
                nc.scalar.dma_start(out=sinn, in_=sinot[tt * P:(tt + 1) * P])
                return cosn, sinn

            for s in range(4):
                wsl = wsl_pre.pop(s, None)
                if wsl is None:
                    wsl = load_slab(wqt, s, f"wq{s}")
                if s + 2 < 4 and (s + 2) not in wsl_pre:
                    wsl_pre[s + 2] = load_slab(wqt, s + 2, f"wq{s + 2}")
                for tt in range(NQT):
                    ps = psS.tile([P, 512], f32, tag="S", name="psq")
                    for kt in range(NCT):
                        nc.tensor.matmul(
                            ps,
                            xoT[kt // 4][:, kt % 4, tt * P:(tt + 1) * P],
                            wsl[:, kt, :],
                            start=(kt == 0),
                            stop=(kt == NCT - 1),
                        )
                    # stage psum out via the idle ACT engine so the psum slot
                    # recycles at matmul speed (rope reads the SBUF copy)
                    qstage = stgpool.tile([P, 512], f32, tag="qstage",
                                          name=f"qst{s}{tt}")
                    nc.scalar.copy(qstage, ps)
                    cos4, sin4 = cos_tiles(tt, f"q{s}{tt}")
                    qhat = qhpool.tile([P, 4, HD], bf16, tag="qhat", name="qhat")
                    rope_rms(qstage.rearrange("p (h d) -> p h d", h=4),
                             cos4, sin4, qhat, eps_q, 1.0 / HD, 4)
                    drain_pending(1)
                    pending.append(
                        lambda qhat=qhat, s=s, tt=tt: pack_transpose(
                            qhat,
                            qT[:, 4 * s:4 * s + 4, (3 - tt) * P:(4 - tt) * P],
                            4,
                        )
                    )
            drain_pending()

            # ---------------- phase 2: attention (scores-transposed) -------
            # q-slot columns are stored high-slot-first: the still-valid slots
            # for key tile kt are columns [0, n) with n = 512 - 128*(kt//4).
            # prefetch the first two wo slabs; their DMAs run under phase 2
            w3s = {0: load_slab(wot, 0, "wo0"), 1: load_slab(wot, 1, "wo1")}

            tail_state = []  # (yt_psum, den_psum, h)

            def emit_tail():
                if not tail_state:
                    return
                yt, den, h = tail_state.pop(0)
                rinv = smallf.tile([P, QROWS], f32, tag="rq", name="rqinv")
                nc.vector.reciprocal_approx_fast(rinv, den)
                nc.vector.tensor_tensor(yT[:, h, :], yt, rinv, op=OP.mult)

            # Two-pack-lagged pipeline so the in-order PE sequencer never
            # waits: PV/presum for pack i-2 (its exp is long done), then the
            # S-fill of pack i; den matmuls lag one further flush so the DVE
            # presum adds are also done when PE reaches them.
            work = []  # (st, pt_tile, slices) awaiting PV/presum
            denq = []  # delayed denominator matmuls

            def drain_denq():
                while denq:
                    den, acc, ng, first, last = denq.pop(0)
                    nc.tensor.matmul(
                        den[:, 0:ng], ones128, acc,
                        start=first, stop=last, skip_group_check=True,
                    )

            def flush_one():
                drain_denq()
                st, ptt, slices = work.pop(0)
                for (off, kt, n) in slices:
                    sl = ptt[:, off:off + n]
                    nc.tensor.matmul(
                        st["yt"][:, 0:n], vAs[st["kvh"]][:, kt, :], sl,
                        start=(st["cnt"] == 0), stop=(st["cnt"] == NT - 1),
                        skip_group_check=True,
                    )
                    st["cnt"] += 1
                    grp = kt // 4
                    if grp not in st["accs"]:
                        st["accs"][grp] = sl
                    else:
                        nc.vector.tensor_tensor(
                            st["accs"][grp], st["accs"][grp], sl, op=OP.add)
                    st["gleft"][grp] -= 1
                    if st["gleft"][grp] == 0:
                        if st["den"] is None:
                            st["den"] = psD.tile([P, QROWS], f32, tag="D",
                                                 name=f"den{st['h']}")
                        denq.append((st["den"], st["accs"][grp],
                                     512 - 128 * grp,
                                     st["dcnt"] == 0, st["dcnt"] == 3))
                        st["dcnt"] += 1
                if st["cnt"] == NT:
                    tail_state.append((st["yt"], st["den"], st["h"]))

            for h in range(NH):
                kvh = h // (NH // NKV)
                st = {"h": h, "kvh": kvh, "cnt": 0, "dcnt": 0,
                      "yt": psY.tile([P, QROWS], f32, tag="Y", name=f"yt{h}"),
                      "den": None, "accs": {},
                      "gleft": {0: 4, 1: 4, 2: 4, 3: 4}}
                for ti, entries in enumerate(PACK):
                    if len(work) >= 2:
                        flush_one()
                    if ti == 2 and tail_state:
                        emit_tail()
                    W = sum(e[2] for e in entries)
                    S = psS.tile([P, 1024], f32, tag="S", name=f"Sp{h}{ti}")
                    for (off, kt, n) in entries:
                        # columns [0, n-128) are fully-valid chunks; the last
                        # 128 (own chunk) get the additive causal bias via a
                        # second accumulating matmul (identity stationary).
                        if n > P:
                            nc.tensor.matmul(
                                S[:, off:off + n - P],
                                kTs[kvh][:, kt * P:(kt + 1) * P],
                                qT[:, h, 0:n - P],
                                start=True, stop=True,
                                skip_group_check=True,
                            )
                        nc.tensor.matmul(
                            S[:, off + n - P:off + n],
                            kTs[kvh][:, kt * P:(kt + 1) * P],
                            qT[:, h, n - P:n],
                            start=True, stop=False,
                            skip_group_check=True,
                        )
                        nc.tensor.matmul(
                            S[:, off + n - P:off + n],
                            ident,
                            qmask[:, kt // 4, kt % 4, :],
                            start=False, stop=True,
                            skip_group_check=True,
                        )
                    ptt = ptpool.tile([P, 1024], bf16, tag="pt", name="pt")
                    # attn scale already folded into k's rms normalization
                    nc.scalar.activation(ptt[:, 0:W], S[:, 0:W], AF.Exp, scale=1.0)
                    work.append((st, ptt, entries))
            while work:
                flush_one()
            drain_denq()
            while tail_state:
                emit_tail()

            # ---------------- phase 3: output projection ----------------
            for s3 in range(4):
                w3 = w3s.pop(s3)
                if s3 + 2 < 4:
                    w3s[s3 + 2] = load_slab(wot, s3 + 2, f"wo{s3 + 2}")
                for qt in range(4):
                    # alternate psum pools so four O tiles are in flight and
                    # the PE never waits on the ot copy draining a slot
                    ps = (psS if qt % 2 == 0 else psD).tile(
                        [P, 512], f32, tag="S" if qt % 2 == 0 else "D",
                        name="ps3")
                    for ct in range(NCT):
                        nc.tensor.matmul(
                            ps,
                            yT[:, ct, (3 - qt) * P:(4 - qt) * P],
                            w3[:, ct, :],
                            start=(ct == 0),
                            stop=(ct == NCT - 1),
                        )
                    ot = outpool.tile([P, 512], bf16, tag="ot", name="ot")
                    nc.vector.tensor_copy(ot, ps)
                    nc.sync.dma_start(
                        out=yo[qt * P:(qt + 1) * P, s3 * 512:(s3 + 1) * 512],
                        in_=ot,
                    )

    nc.compile()
    return nc


def _get_nc():
    if "nc" not in _CACHE:
        _CACHE["nc"] = _build()
    return _CACHE["nc"]


def _tile_x(xT):
    """[C, T] -> [8*4*128, 4, 256]: (e, gr, p) major; row gr*512+a*128+p,
    col e*256+n lands at [e*4*128 + gr*128 + p, a, n]."""
    t = xT.reshape(4, 4, P, 8, 256).transpose(3, 0, 2, 1, 4)
    return np.ascontiguousarray(t.reshape(8 * 4 * P, 4, 256))


def _tile_w(w):
    """[C, C] -> [4*128, 16, 512]: slab-major; row a*128+p, col s*512+n
    lands at [s*128 + p, a, n]."""
    t = w.reshape(NCT, P, 4, 512).transpose(2, 1, 0, 3)
    return np.ascontiguousarray(t.reshape(4 * P, NCT, 512))


def _tile_xo(xoT):
    """[C, 512] -> [4*128, 4, 512]: row gr*512+a*128+p at [gr*128+p, a, :]."""
    t = xoT.reshape(4, 4, P, QROWS).transpose(0, 2, 1, 3)
    return np.ascontiguousarray(t.reshape(4 * P, 4, QROWS))


def _tile_cos_k(cosr):
    """[T, 64] -> [4*128, 4, 64]: token q*512+a*128+p at [q*128+p, a, :]."""
    t = cosr.reshape(4, 4, P, 64).transpose(0, 2, 1, 3)
    return np.ascontiguousarray(t.reshape(4 * P, 4, 64))


def _tile_cos_q(cosg):
    """[512, 64] -> [4*128, 4, 64]: token tt*128+p at [tt*128+p, h, :],
    replicated across the 4 heads of a q slab."""
    t = np.repeat(cosg.reshape(4, P, 1, 64), 4, axis=2)
    return np.ascontiguousarray(t.reshape(4 * P, 4, 64))


def _in_maps(x, cosr, sinr, wq, wk, wv, wo):
    xTb = [_tile_x(np.ascontiguousarray(x[b].T).astype(BF)) for b in range(B)]
    wqb = _tile_w(wq.astype(BF))
    wob = _tile_w(wo.astype(BF))
    cosk = _tile_cos_k(cosr)
    sink = _tile_cos_k(sinr)
    maps = []
    for core in range(8):
        b, g = core // 4, core % 4
        rows = _rows(g)
        wkv = np.concatenate(
            [wk[:, g * HD:(g + 1) * HD], wv[:, g * HD:(g + 1) * HD]],
            axis=1).astype(BF)
        maps.append({
            "xet": xTb[b],
            "xot": _tile_xo(x[b][rows].T.astype(BF)),
            "cosft": cosk,
            "sinft": sink,
            "cosot": _tile_cos_q(cosr[rows]),
            "sinot": _tile_cos_q(sinr[rows]),
            "wqt": wqb,
            "wkvt": np.ascontiguousarray(
                wkv.reshape(NCT, P, 256).transpose(1, 0, 2)),
            "wot": wob,
            "qmt": _qbias_t(g),
        })
    return maps


def kernel(x, cos, sin, wq, wk, wv, wo):
    from concourse.bass_utils import run_bass_kernel_spmd

    x = np.ascontiguousarray(np.asarray(x, np.float32))
    cosr = np.ascontiguousarray(np.asarray(cos, np.float32).reshape(T, HD // 2))
    sinr = np.ascontiguousarray(np.asarray(sin, np.float32).reshape(T, HD // 2))
    wq = np.ascontiguousarray(np.asarray(wq, np.float32))
    wk = np.ascontiguousarray(np.asarray(wk, np.float32))
    wv = np.ascontiguousarray(np.asarray(wv, np.float32))
    wo = np.ascontiguousarray(np.asarray(wo, np.float32))

    nc = _get_nc()
    maps = _in_maps(x, cosr, sinr, wq, wk, wv, wo)
    _CACHE["in_maps"] = maps
    res = run_bass_kernel_spmd(nc, maps, list(range(8)))
    y = np.empty((B, T, C), np.float32)
    for core in range(8):
        b, g = core // 4, core % 4
        y[b][_rows(g)] = res.results[core]["yo"].astype(np.float32)
    return y
